# revision 13
# baseline (speedup 1.0000x reference)
"""Causal self-attention (B=2, T=2048, C=1024, H=16, D=64) on 8 trn2 cores.

Sharding: tensor-parallel on heads — 2 heads per core. Each core computes
QKV projection for its 2 heads, causal softmax attention, and its heads'
slice of the output projection (a rank-128 partial sum of the full output).
The host pre-transposes x to [B, C, T], slices the weights per core, and
sums the 8 partial outputs (+ proj bias + v-bias@proj_w correction).

Bias algebra (exact):
  - k bias: softmax rows are shift-invariant, so S = (q+bq). k_raw modulo
    per-row constants — the k bias is dropped entirely on device.
  - v bias: rows of normalized attention sum to 1, so attn@(V+1 bv^T) =
    attn@V + 1 bv^T; the bv@proj_w term is a constant row added on host.
  Only the q bias remains on device (fused into the q PSUM->SBUF activation).

Device kernel layout (per core):
  - x^T chunks [128(C), T] stream from DRAM; QKV computed as W^T @ x^T
    giving q/k/v in [feat, tok] layout. All matmul inputs bf16, fp32 accum.
  - Attention q-tiles of 512 tokens. Per k-chunk (128 tokens) the TWO
    heads' S^T matmuls are emitted back-to-back: K=64 contractions at
    base partitions 0/64 auto-derive tile_position (0,0)/(64,0), so they
    run CONCURRENTLY in the two row-halves of the PE array (row tiling).
  - One fused exp ACTIVATE per (q-tile, k-chunk) covers both heads via a
    [128, 2, 512] PSUM pair tile (halves ScalarE instruction overhead).
  - O^T accumulates as (V|1)-chunk.T @ P^T; the ones column makes row 64
    the softmax denominator. Causality: subtile skipping + one [128,128]
    triangle mask multiply per head on the diagonal chunk.
  - Projection: single K=128 matmul per output tile.
  - HAM warm-start: a burst of junk matmuls on the identity tile runs
    during the initial x-DMA wait so real work starts at 2.4 GHz, and a
    filler queue (QKV/V-transpose/projection thunks) keeps the PE dense
    through the ScalarE-paced attention inner loop.
"""

from collections import deque

import numpy as np

import concourse.bass as bass
import concourse.tile as tile
from concourse import bacc, mybir
from concourse.bass_utils import run_bass_kernel_spmd

dt = mybir.dt
AF = mybir.ActivationFunctionType

B, T, C, H, D = 2, 2048, 1024, 16, 64
NCORES = 8
HPC = H // NCORES          # heads per core = 2
QT = 512                   # q-tile (columns of S^T/O^T psum tiles)
KC = 128                   # k chunk (partition dim of S^T)
SCALE = 1.0 / 8.0          # 1/sqrt(D)
NWARM = 40                 # junk matmuls to warm the HAM clock gate

_CACHE = {}


def _emit(tc):
    from contextlib import ExitStack
    with ExitStack() as ctx:
        _emit_body(tc, ctx)


def _emit_body(tc, ctx):
    nc = tc.nc
    f32, bf16 = dt.float32, dt.bfloat16

    # x pre-chunked on host as [B, chunk, 128, plane, tok]: per-partition
    # contiguous 8KB runs per DMA (vs 256B packets from a strided gather)
    xT = nc.dram_tensor("xT", [B, 4, 128, 8, QT], bf16,
                        kind="ExternalInput").ap()
    wqkv = nc.dram_tensor("wqkv", [128, 8, 384], bf16,
                          kind="ExternalInput").ap()
    bq = nc.dram_tensor("bq", [128, 1], f32, kind="ExternalInput").ap()
    wp = nc.dram_tensor("wp", [128, C], bf16, kind="ExternalInput").ap()
    tri = nc.dram_tensor("tri", [128, 128], bf16, kind="ExternalInput").ap()
    ident = nc.dram_tensor("ident", [128, 128], bf16, kind="ExternalInput").ap()
    outp = nc.dram_tensor("outp", [B, T, C], f32, kind="ExternalOutput").ap()

    consts = ctx.enter_context(tc.tile_pool(name="consts", bufs=1))
    xpool = ctx.enter_context(tc.tile_pool(name="xpool", bufs=2))
    qkvpool = ctx.enter_context(tc.tile_pool(name="qkvpool", bufs=6))
    vtmpool = ctx.enter_context(tc.tile_pool(name="vtmpool", bufs=2))
    ptpool = ctx.enter_context(tc.tile_pool(name="ptpool", bufs=4))
    unormp = ctx.enter_context(tc.tile_pool(name="unormp", bufs=3))
    orawp = ctx.enter_context(tc.tile_pool(name="orawp", bufs=4))
    rows = ctx.enter_context(tc.tile_pool(name="rows", bufs=4))
    outsb = ctx.enter_context(tc.tile_pool(name="outsb", bufs=8))
    stp = ctx.enter_context(tc.tile_pool(name="stp", bufs=2, space="PSUM"))
    otp = ctx.enter_context(tc.tile_pool(name="otp", bufs=2, space="PSUM"))
    miscp = ctx.enter_context(tc.tile_pool(name="miscp", bufs=2, space="PSUM"))

    # HAM warm-up: junk matmuls gated only on a local memset so they start
    # right after the engine preamble, covering the whole x-DMA wait.
    wu_in = consts.tile([128, 128], bf16, tag="wuin")
    nc.vector.memset(wu_in, 1.0)
    wu = miscp.tile([128, 128], f32, tag="misc", name="wu")
    for _ in range(NWARM):
        nc.tensor.matmul(wu[:, :], wu_in[:, :], wu_in[:, :],
                         start=True, stop=True)

    # constants / weights resident in SBUF. Order matters: the sync DMA
    # queue serializes, so the first x chunk goes right after the tensors
    # needed by the first QKV chain; wp (first needed by the projection
    # ~30us in) and the remaining x chunks follow.
    tri_sb = consts.tile([128, 128], bf16, tag="tri")
    nc.sync.dma_start(out=tri_sb, in_=tri)
    id_sb = consts.tile([128, 128], bf16, tag="id")
    nc.sync.dma_start(out=id_sb, in_=ident)
    w_sb = consts.tile([128, 8, 384], bf16, tag="w")
    nc.sync.dma_start(out=w_sb, in_=wqkv)

    xps = [xpool.tile([128, 4, 8, QT], bf16, tag="xp", name=f"xp{b}")
           for b in range(B)]

    def dma_x(b, tg):
        nc.sync.dma_start(out=xps[b][:, tg, :, :], in_=xT[b, tg])

    dma_x(0, 0)
    bq_sb = consts.tile([128, 1], f32, tag="b")
    nc.sync.dma_start(out=bq_sb, in_=bq)
    wp_sb = consts.tile([128, C], bf16, tag="wp")
    nc.sync.dma_start(out=wp_sb, in_=wp)
    for tg in range(1, 4):
        dma_x(0, tg)
    for tg in range(4):
        dma_x(1, tg)

    # dummy exp front-loads the ~1.3us ACT table load
    act_wu = rows.tile([128, 1], f32, tag="actwu")
    nc.scalar.activation(act_wu[:, :], bq_sb[:, :], AF.Exp, scale=SCALE)

    # per-batch q/k/v in [feat, tok] layout and V in token-major layout
    qkv_t = {b: [qkvpool.tile([128, T], bf16, tag="qkv", name=f"qkv{b}_{m}")
                 for m in range(3)] for b in range(B)}
    vt_t = {b: vtmpool.tile([128, 16, HPC * 65], bf16, tag="vtm",
                            name=f"vt{b}")
            for b in range(B)}

    def th_qkv(b, tg, m):
        """One QKV chain: 8 accumulating matmuls + PSUM->SBUF move."""
        def th():
            pg = miscp.tile([128, QT], f32, tag="misc", name="pg")
            t0 = tg * QT
            for kcw in range(8):
                nc.tensor.matmul(
                    pg[:, :],
                    w_sb[:, kcw, 128 * m:128 * m + 128],
                    xps[b][:, tg, kcw, :],
                    start=(kcw == 0), stop=(kcw == 7),
                )
            dst = qkv_t[b][m]
            if m == 0:  # q: fused bias
                nc.scalar.activation(dst[:, t0:t0 + QT], pg[:, :],
                                     AF.Identity, bias=bq_sb[:, :])
            else:       # k, v: biases dropped (host-folded)
                nc.vector.tensor_copy(out=dst[:, t0:t0 + QT], in_=pg[:, :])
        return th

    def th_ones(b):
        def th():
            nc.vector.memset(
                vt_t[b].rearrange("p k (h c) -> p k h c", h=HPC)[:, :, :, 64:65],
                1.0)
        return th

    def th_vt(b, j0, nj):
        """V [feat,tok] -> token-major [128, j, (h 65)] via PE transposes."""
        def th():
            vT_t, vt = qkv_t[b][2], vt_t[b]
            for j in range(j0, j0 + nj):
                tp = miscp.tile([128, 128], bf16, tag="misc", name="tp")
                nc.tensor.transpose(
                    tp[:, :], vT_t[:, 128 * j:128 * j + 128], id_sb[:, :])
                nc.vector.tensor_copy(
                    out=vt[:, j, :].rearrange(
                        "p (h c) -> p h c", h=HPC)[:, :, 0:64],
                    in_=tp.rearrange("p (h c) -> p h c", h=HPC),
                )
        return th

    def th_proj(b, q0, un, ts, ct):
        """Projection of one [128 tok, 512 feat] output tile + store."""
        def th():
            a0 = q0 + ts * 128
            pp = miscp.tile([128, 512], f32, tag="misc", name="pp")
            nc.tensor.matmul(
                pp[:, :],
                un[:, ts * 128:(ts + 1) * 128],
                wp_sb[:, ct * 512:(ct + 1) * 512],
                start=True, stop=True,
            )
            ob = outsb.tile([128, 512], f32, tag="osb")
            nc.vector.tensor_copy(out=ob[:, :], in_=pp[:, :])
            nc.sync.dma_start(
                out=outp[b, a0:a0 + 128, ct * 512:(ct + 1) * 512],
                in_=ob[:, :])
        return th

    # front fillers carry (b, tg) tags so attention can force-drain deps
    front = deque()
    projq = deque()
    for b in range(B):
        tgs = range(1, 4) if b == 0 else range(4)
        if b == 1:
            front.append((1, 0, th_ones(1)))
        for tg in tgs:
            front.append((b, tg, th_qkv(b, tg, 1)))   # k
            front.append((b, tg, th_qkv(b, tg, 0)))   # q
            front.append((b, tg, th_qkv(b, tg, 2)))   # v
            front.append((b, tg, th_vt(b, tg * 4, 2)))
            front.append((b, tg, th_vt(b, tg * 4 + 2, 2)))

    slots_left = [2 * sum(4 * (qt + 1) for qt in range(4))]  # 80 kc slots

    def pop_filler():
        # prefer deferred projections; keep front (b1 QKV) as the PE-work
        # reserve for the batch transition where attention runs thin
        if projq:
            projq.popleft()()
        elif front:
            front.popleft()[2]()

    def force_front(b, qt):
        while front and (front[0][0] < b or
                         (front[0][0] == b and front[0][1] <= qt)):
            front.popleft()[2]()

    # batch 0 / tile-group 0 front work runs densely right away
    th_ones(0)()
    th_qkv(0, 0, 1)(); th_qkv(0, 0, 0)(); th_qkv(0, 0, 2)()
    th_vt(0, 0, 2)(); th_vt(0, 2, 2)()

    for b in range(B):
        qT_t, kT_t, vT_t = qkv_t[b]
        vt = vt_t[b]
        for qt in range(T // QT):
            q0 = qt * QT
            nkc = (q0 + QT) // KC
            force_front(b, qt)
            ots = [otp.tile([65, QT], f32, tag="ot", name=f"ot{h}")
                   for h in range(HPC)]
            for kc in range(nkc):
                k0 = kc * KC
                ls = max(0, k0 - q0)
                diag = k0 >= q0
                st = stp.tile([128, HPC, QT], f32, tag="st")
                for h in range(HPC):
                    nc.tensor.matmul(
                        st[:, h, ls:QT],
                        kT_t[64 * h:64 * h + 64, k0:k0 + KC],
                        qT_t[64 * h:64 * h + 64, q0 + ls:q0 + QT],
                        start=True, stop=True,
                    )
                pt = ptpool.tile([128, HPC, QT], bf16, tag="pt")
                nc.scalar.activation(pt[:, :, ls:QT], st[:, :, ls:QT],
                                     AF.Exp, scale=SCALE)
                if diag:
                    for h in range(HPC):
                        nc.vector.tensor_mul(
                            pt[:, h, ls:ls + 128], pt[:, h, ls:ls + 128],
                            tri_sb[:, :])
                # start=True only on the first MM into each ot bank (it
                # clears the whole bank's has_written bits); stop=True only
                # on the last emitted MM (kc=nkc-1's diagonal strip).
                for h in range(HPC):
                    vch = vt[:, kc, 65 * h:65 * h + 65]
                    if diag:
                        nc.tensor.matmul(
                            ots[h][:, ls:ls + 128], vch,
                            pt[:, h, ls:ls + 128],
                            start=(kc == 0), stop=(kc == nkc - 1),
                        )
                        if ls + 128 < QT:
                            nc.tensor.matmul(
                                ots[h][:, ls + 128:QT], vch,
                                pt[:, h, ls + 128:QT],
                                start=False, stop=False,
                            )
                    else:
                        nc.tensor.matmul(
                            ots[h][:, 0:QT], vch, pt[:, h, 0:QT],
                            start=(kc == 0), stop=False,
                        )
                slots_left[0] -= 1
                need = len(front) + len(projq)
                npop = min(3, max(1, -(-need // max(slots_left[0], 1))))
                for _ in range(npop):
                    pop_filler()

            # drain ot -> SBUF immediately so the PSUM banks free for the
            # next q-tile; the normalize chain then runs off-critical-path.
            oraws, ses = [], []
            for h in range(HPC):
                oraw = orawp.tile([64, QT], f32, tag="oraw", name=f"or{h}")
                nc.vector.tensor_copy(out=oraw[:, :], in_=ots[h][0:64, :])
                se = rows.tile([1, QT], f32, tag="se", name=f"se{h}")
                nc.vector.tensor_copy(out=se[:, :], in_=ots[h][64:65, :])
                oraws.append(oraw)
                ses.append(se)
            un = unormp.tile([128, QT], bf16, tag="un", name=f"un{b}{qt}")
            rbs = []
            for h in range(HPC):
                rc = rows.tile([1, QT], f32, tag="rc", name=f"rc{h}")
                nc.vector.reciprocal_approx_fast(rc[:, :], ses[h][:, :])
                rb = rows.tile([64, QT], f32, tag="rb", name=f"rb{h}")
                nc.gpsimd.partition_broadcast(rb[:, :], rc[:, :])
                rbs.append(rb)
            last = (b == B - 1 and qt == T // QT - 1)
            for ts in range(QT // 128):
                c0, c1 = ts * 128, (ts + 1) * 128
                for h in range(HPC):
                    nc.vector.tensor_mul(
                        un[64 * h:64 * h + 64, c0:c1],
                        oraws[h][:, c0:c1], rbs[h][:, c0:c1])
                for ct in range(2):
                    th = th_proj(b, q0, un, ts, ct)
                    if last:  # fine-grained tail: project as soon as ready
                        th()
                    else:
                        projq.append(th)

    while front or projq:
        pop_filler()


def build():
    if "nc" in _CACHE:
        return _CACHE["nc"]
    nc = bacc.Bacc("TRN2", target_bir_lowering=False, debug=False,
                   num_devices=NCORES)
    with tile.TileContext(nc) as tc:
        _emit(tc)
    nc.compile()
    _CACHE["nc"] = nc
    return nc


def make_in_maps(x, qkv_w, qkv_b, proj_w):
    import ml_dtypes
    bf16 = ml_dtypes.bfloat16
    x = np.asarray(x, dtype=np.float32)
    qkv_w = np.asarray(qkv_w, dtype=np.float32)
    qkv_b = np.asarray(qkv_b, dtype=np.float32)
    proj_w = np.asarray(proj_w, dtype=np.float32)

    # [B,T,C] -> [B, chunk(4), p(128), plane(8), tok(512)] so each chunk
    # DMA has contiguous 8KB per-partition runs
    xT = np.ascontiguousarray(
        x.reshape(B, 4, QT, 8, 128).transpose(0, 1, 4, 3, 2)).astype(bf16)
    tri = (np.arange(128)[None, :] >= np.arange(128)[:, None]).astype(bf16)
    ident = np.eye(128, dtype=bf16)

    in_maps = []
    for c in range(NCORES):
        s = 64 * HPC * c  # first feature row of this core's heads
        wq = qkv_w[:, s:s + 128]
        wk = qkv_w[:, C + s:C + s + 128]
        wv = qkv_w[:, 2 * C + s:2 * C + s + 128]
        wqkv_c = np.concatenate([wq, wk, wv], axis=1)  # [C, 384]
        # -> [p(128), plane(8), 384] matching the SBUF-resident layout
        wqkv_c = np.ascontiguousarray(
            wqkv_c.reshape(8, 128, 384).transpose(1, 0, 2)).astype(bf16)
        bq_c = np.ascontiguousarray(
            qkv_b[s:s + 128].reshape(128, 1).astype(np.float32))
        wp_c = np.ascontiguousarray(proj_w[s:s + 128, :]).astype(bf16)
        in_maps.append({
            "xT": xT, "wqkv": wqkv_c, "bq": bq_c, "wp": wp_c,
            "tri": tri, "ident": ident,
        })
    return in_maps


def kernel(x, qkv_w, qkv_b, proj_w, proj_b, _trace=False):
    nc = build()
    in_maps = make_in_maps(x, qkv_w, qkv_b, proj_w)
    res = run_bass_kernel_spmd(nc, in_maps, core_ids=list(range(NCORES)),
                               trace=_trace)
    acc = np.zeros((B, T, C), dtype=np.float64)
    for c in range(NCORES):
        acc += res.results[c]["outp"].astype(np.float64)
    # host-folded bias terms: proj bias + v-bias @ proj_w (exact)
    bv = np.asarray(qkv_b, dtype=np.float64)[2 * C:]
    acc += bv @ np.asarray(proj_w, dtype=np.float64)
    acc += np.asarray(proj_b, dtype=np.float64)
    out = acc.astype(np.float32)
    _CACHE["last_results"] = res
    return out


# revision 23
# speedup vs baseline: 1.0788x; 1.0788x over previous
"""Causal self-attention (B=2, T=2048, C=1024, H=16, D=64) on 8 trn2 cores.

Sharding: tensor-parallel on heads — 2 heads per core. Each core computes
QKV projection for its 2 heads, causal softmax attention, and its heads'
slice of the output projection (a rank-128 partial sum of the full output).
The host pre-transposes x to [B, C, T], slices the weights per core, and
sums the 8 partial outputs (+ proj bias + v-bias@proj_w correction).

Bias algebra (exact):
  - k bias: softmax rows are shift-invariant, so S = (q+bq). k_raw modulo
    per-row constants — the k bias is dropped entirely on device.
  - v bias: rows of normalized attention sum to 1, so attn@(V+1 bv^T) =
    attn@V + 1 bv^T; the bv@proj_w term is a constant row added on host.
  Only the q bias remains on device (fused into the q PSUM->SBUF activation).

Device kernel layout (per core):
  - x^T chunks [128(C), T] stream from DRAM; QKV computed as W^T @ x^T
    giving q/k/v in [feat, tok] layout. All matmul inputs bf16, fp32 accum.
  - Attention q-tiles of 512 tokens. Per k-chunk (128 tokens) the TWO
    heads' S^T matmuls are emitted back-to-back: K=64 contractions at
    base partitions 0/64 auto-derive tile_position (0,0)/(64,0), so they
    run CONCURRENTLY in the two row-halves of the PE array (row tiling).
  - One fused exp ACTIVATE per (q-tile, k-chunk) covers both heads via a
    [128, 2, 512] PSUM pair tile (halves ScalarE instruction overhead).
  - O^T accumulates as (V|1)-chunk.T @ P^T; the ones column makes row 64
    the softmax denominator. Causality: subtile skipping + one [128,128]
    triangle mask multiply per head on the diagonal chunk.
  - Projection: single K=128 matmul per output tile.
  - HAM warm-start: a burst of junk matmuls on the identity tile runs
    during the initial x-DMA wait so real work starts at 2.4 GHz, and a
    filler queue (QKV/V-transpose/projection thunks) keeps the PE dense
    through the ScalarE-paced attention inner loop.
"""

from collections import deque

import numpy as np

import concourse.bass as bass
import concourse.tile as tile
from concourse import bacc, mybir
from concourse.bass_utils import run_bass_kernel_spmd

dt = mybir.dt
AF = mybir.ActivationFunctionType

B, T, C, H, D = 2, 2048, 1024, 16, 64
NCORES = 8
HPC = H // NCORES          # heads per core = 2
QT = 512                   # q-tile (columns of S^T/O^T psum tiles)
KC = 128                   # k chunk (partition dim of S^T)
SCALE = 1.0 / 8.0          # 1/sqrt(D)
NWARM = 40                 # junk matmuls to warm the HAM clock gate

_CACHE = {}


def _emit(tc):
    from contextlib import ExitStack
    with ExitStack() as ctx:
        _emit_body(tc, ctx)


def _emit_body(tc, ctx):
    nc = tc.nc
    f32, bf16 = dt.float32, dt.bfloat16

    fp8 = dt.float8e4
    # x pre-chunked on host as [B, chunk, 128, plane, tok]: per-partition
    # contiguous runs per DMA (vs 256B packets from a strided gather).
    # Q/K projections run as fp8 DoubleRow matmuls (w pre-scaled x16 on
    # host; the 1/16 de-scales fold into bq and the exp scale) — softmax
    # attenuates the logit quantization noise. The V projection stays
    # bf16 (its error passes straight through to the output), using a
    # bf16 copy of x and bf16 weights.
    xT = nc.dram_tensor("xT", [B, 4, 128, 8, QT], fp8,
                        kind="ExternalInput").ap()
    xTb = nc.dram_tensor("xTb", [B, 4, 128, 8, QT], bf16,
                         kind="ExternalInput").ap()
    wqkv = nc.dram_tensor("wqkv", [128, 8, 256], fp8,
                          kind="ExternalInput").ap()
    wv = nc.dram_tensor("wv", [128, 8, 128], bf16,
                        kind="ExternalInput").ap()
    bq = nc.dram_tensor("bq", [128, 1], f32, kind="ExternalInput").ap()
    wp = nc.dram_tensor("wp", [128, C], bf16, kind="ExternalInput").ap()
    tri = nc.dram_tensor("tri", [128, 128], bf16, kind="ExternalInput").ap()
    ident = nc.dram_tensor("ident", [128, 128], bf16, kind="ExternalInput").ap()
    outp = nc.dram_tensor("outp", [B, T, C], bf16, kind="ExternalOutput").ap()

    consts = ctx.enter_context(tc.tile_pool(name="consts", bufs=1))
    xpool = ctx.enter_context(tc.tile_pool(name="xpool", bufs=2))
    qkvpool = ctx.enter_context(tc.tile_pool(name="qkvpool", bufs=6))
    vtmpool = ctx.enter_context(tc.tile_pool(name="vtmpool", bufs=2))
    ptpool = ctx.enter_context(tc.tile_pool(name="ptpool", bufs=4))
    unormp = ctx.enter_context(tc.tile_pool(name="unormp", bufs=3))
    orawp = ctx.enter_context(tc.tile_pool(name="orawp", bufs=4))
    rows = ctx.enter_context(tc.tile_pool(name="rows", bufs=4))
    outsb = ctx.enter_context(tc.tile_pool(name="outsb", bufs=8))
    stp = ctx.enter_context(tc.tile_pool(name="stp", bufs=2, space="PSUM"))
    otp = ctx.enter_context(tc.tile_pool(name="otp", bufs=2, space="PSUM"))
    miscp = ctx.enter_context(tc.tile_pool(name="miscp", bufs=2, space="PSUM"))

    # HAM warm-up: junk matmuls gated only on a local memset so they start
    # right after the engine preamble, covering the whole x-DMA wait.
    wu_in = consts.tile([128, 128], bf16, tag="wuin")
    nc.vector.memset(wu_in, 1.0)
    wu = miscp.tile([128, 128], f32, tag="misc", name="wu")
    for _ in range(NWARM):
        nc.tensor.matmul(wu[:, :], wu_in[:, :], wu_in[:, :],
                         start=True, stop=True)

    # constants / weights resident in SBUF. Order matters: the sync DMA
    # queue serializes, so the first x chunk goes right after the tensors
    # needed by the first QKV chain; wp (first needed by the projection
    # ~30us in) and the remaining x chunks follow.
    tri_sb = consts.tile([128, 128], bf16, tag="tri")
    nc.sync.dma_start(out=tri_sb, in_=tri)
    id_sb = consts.tile([128, 128], bf16, tag="id")
    nc.sync.dma_start(out=id_sb, in_=ident)
    w_sb = consts.tile([128, 8, 256], fp8, tag="w")
    nc.sync.dma_start(out=w_sb, in_=wqkv)
    wv_sb = consts.tile([128, 8, 128], bf16, tag="wv")
    nc.sync.dma_start(out=wv_sb, in_=wv)

    xps = [xpool.tile([128, 4, 8, QT], fp8, tag="xp", name=f"xp{b}")
           for b in range(B)]
    xpbs = [xpool.tile([128, 4, 8, QT], bf16, tag="xpb", name=f"xpb{b}")
            for b in range(B)]

    def dma_x(b, tg):
        nc.sync.dma_start(out=xps[b][:, tg, :, :], in_=xT[b, tg])
        nc.sync.dma_start(out=xpbs[b][:, tg, :, :], in_=xTb[b, tg])

    dma_x(0, 0)
    bq_sb = consts.tile([128, 1], f32, tag="b")
    nc.sync.dma_start(out=bq_sb, in_=bq)
    wp_sb = consts.tile([128, C], bf16, tag="wp")
    nc.sync.dma_start(out=wp_sb, in_=wp)
    for tg in range(1, 4):
        dma_x(0, tg)
    for tg in range(4):
        dma_x(1, tg)

    # dummy exp front-loads the ~1.3us ACT table load
    act_wu = rows.tile([128, 1], f32, tag="actwu")
    nc.scalar.activation(act_wu[:, :], bq_sb[:, :], AF.Exp, scale=SCALE)

    # per-batch q/k/v in [feat, tok] layout and V in token-major layout
    qkv_t = {b: [qkvpool.tile([128, T], bf16, tag="qkv", name=f"qkv{b}_{m}")
                 for m in range(3)] for b in range(B)}
    vt_t = {b: vtmpool.tile([128, 16, HPC * 65], bf16, tag="vtm",
                            name=f"vt{b}")
            for b in range(B)}

    def th_qkv(b, tg, m):
        """One QKV chain: 4 fp8 DoubleRow matmuls + PSUM->SBUF move.
        All PSUM->SBUF moves run on ScalarE (keeps the DVE FIFO clear of
        the attention inner loop's dependencies)."""
        def th():
            pg = miscp.tile([128, QT], f32, tag="misc", name="pg")
            t0 = tg * QT
            if m == 2:  # v: bf16
                for kcw in range(8):
                    nc.tensor.matmul(
                        pg[:, :], wv_sb[:, kcw, :], xpbs[b][:, tg, kcw, :],
                        start=(kcw == 0), stop=(kcw == 7),
                    )
            else:       # q, k: fp8 DoubleRow
                for kcw in range(0, 8, 2):
                    nc.tensor.matmul(
                        pg[:, :],
                        w_sb[:, kcw:kcw + 2, 128 * m:128 * m + 128],
                        xps[b][:, tg, kcw:kcw + 2, :],
                        start=(kcw == 0), stop=(kcw == 6),
                        perf_mode=mybir.MatmulPerfMode.DoubleRow,
                    )
            dst = qkv_t[b][m]
            if m == 0:  # q: fused bias (x16, matching the x16 weights)
                nc.scalar.activation(dst[:, t0:t0 + QT], pg[:, :],
                                     AF.Identity, bias=bq_sb[:, :])
            else:       # k, v: biases dropped (host-folded)
                nc.scalar.copy(dst[:, t0:t0 + QT], pg[:, :])
        return th

    def th_ones(b):
        def th():
            nc.vector.memset(
                vt_t[b].rearrange("p k (h c) -> p k h c", h=HPC)[:, :, :, 64:65],
                1.0)
        return th

    def th_vt(b, j0, nj):
        """V [feat,tok] -> token-major [128, j, (h 65)] via PE transposes."""
        def th():
            vT_t, vt = qkv_t[b][2], vt_t[b]
            for j in range(j0, j0 + nj):
                tp = miscp.tile([128, 128], bf16, tag="misc", name="tp")
                nc.tensor.transpose(
                    tp[:, :], vT_t[:, 128 * j:128 * j + 128], id_sb[:, :])
                nc.vector.tensor_copy(
                    out=vt[:, j, :].rearrange(
                        "p (h c) -> p h c", h=HPC)[:, :, 0:64],
                    in_=tp.rearrange("p (h c) -> p h c", h=HPC),
                )
        return th

    def th_proj(b, q0, un, ts, ct):
        """Projection of one [128 tok, 512 feat] output tile + store."""
        def th():
            a0 = q0 + ts * 128
            pp = miscp.tile([128, 512], f32, tag="misc", name="pp")
            nc.tensor.matmul(
                pp[:, :],
                un[:, ts * 128:(ts + 1) * 128],
                wp_sb[:, ct * 512:(ct + 1) * 512],
                start=True, stop=True,
            )
            ob = outsb.tile([128, 512], bf16, tag="osb")
            if ct == 0:  # split PSUM->SBUF copies across ScalarE and DVE
                nc.scalar.copy(ob[:, :], pp[:, :])
            else:
                nc.vector.tensor_copy(out=ob[:, :], in_=pp[:, :])
            nc.sync.dma_start(
                out=outp[b, a0:a0 + 128, ct * 512:(ct + 1) * 512],
                in_=ob[:, :])
        return th

    # front fillers carry (b, tg) tags so attention can force-drain deps
    front = deque()
    projq = deque()
    for b in range(B):
        tgs = range(1, 4) if b == 0 else range(4)
        if b == 1:
            front.append((1, 0, th_ones(1)))
        for tg in tgs:
            front.append((b, tg, th_qkv(b, tg, 1)))   # k
            front.append((b, tg, th_qkv(b, tg, 0)))   # q
            front.append((b, tg, th_qkv(b, tg, 2)))   # v
            front.append((b, tg, th_vt(b, tg * 4, 2)))
            front.append((b, tg, th_vt(b, tg * 4 + 2, 2)))

    slots_left = [2 * sum(4 * (qt + 1) for qt in range(4))]  # 80 kc slots

    def pop_filler():
        # prefer deferred projections; keep front (b1 QKV) as the PE-work
        # reserve for the batch transition where attention runs thin
        if projq:
            projq.popleft()()
        elif front:
            front.popleft()[2]()

    def force_front(b, qt):
        while front and (front[0][0] < b or
                         (front[0][0] == b and front[0][1] <= qt)):
            front.popleft()[2]()

    # batch 0 / tile-group 0 front work runs densely right away
    th_ones(0)()
    th_qkv(0, 0, 1)(); th_qkv(0, 0, 0)(); th_qkv(0, 0, 2)()
    th_vt(0, 0, 2)(); th_vt(0, 2, 2)()

    for b in range(B):
        qT_t, kT_t, vT_t = qkv_t[b]
        vt = vt_t[b]
        for qt in range(T // QT):
            q0 = qt * QT
            nkc = (q0 + QT) // KC
            force_front(b, qt)
            ots = [otp.tile([65, QT], f32, tag="ot", name=f"ot{h}")
                   for h in range(HPC)]
            for kc in range(nkc):
                k0 = kc * KC
                ls = max(0, k0 - q0)
                diag = k0 >= q0
                st = stp.tile([128, HPC, QT], f32, tag="st")
                for h in range(HPC):
                    nc.tensor.matmul(
                        st[:, h, ls:QT],
                        kT_t[64 * h:64 * h + 64, k0:k0 + KC],
                        qT_t[64 * h:64 * h + 64, q0 + ls:q0 + QT],
                        start=True, stop=True,
                    )
                pt = ptpool.tile([128, HPC, QT], bf16, tag="pt")
                nc.scalar.activation(pt[:, :, ls:QT], st[:, :, ls:QT],
                                     AF.Exp, scale=SCALE / 256.0)
                if diag:
                    for h in range(HPC):
                        nc.vector.tensor_mul(
                            pt[:, h, ls:ls + 128], pt[:, h, ls:ls + 128],
                            tri_sb[:, :])
                # start=True only on the first MM into each ot bank (it
                # clears the whole bank's has_written bits); stop=True only
                # on the last emitted MM (kc=nkc-1's diagonal strip).
                for h in range(HPC):
                    vch = vt[:, kc, 65 * h:65 * h + 65]
                    if diag:
                        nc.tensor.matmul(
                            ots[h][:, ls:ls + 128], vch,
                            pt[:, h, ls:ls + 128],
                            start=(kc == 0), stop=(kc == nkc - 1),
                        )
                        if ls + 128 < QT:
                            nc.tensor.matmul(
                                ots[h][:, ls + 128:QT], vch,
                                pt[:, h, ls + 128:QT],
                                start=False, stop=False,
                            )
                    else:
                        nc.tensor.matmul(
                            ots[h][:, 0:QT], vch, pt[:, h, 0:QT],
                            start=(kc == 0), stop=False,
                        )
                slots_left[0] -= 1
                need = len(front) + len(projq)
                npop = min(3, max(1, -(-need // max(slots_left[0], 1))))
                for _ in range(npop):
                    pop_filler()

            # drain ot -> SBUF immediately so the PSUM banks free for the
            # next q-tile; the normalize chain then runs off-critical-path.
            oraws, ses = [], []
            for h in range(HPC):
                oraw = orawp.tile([64, QT], f32, tag="oraw", name=f"or{h}")
                nc.vector.tensor_copy(out=oraw[:, :], in_=ots[h][0:64, :])
                se = rows.tile([1, QT], f32, tag="se", name=f"se{h}")
                nc.vector.tensor_copy(out=se[:, :], in_=ots[h][64:65, :])
                oraws.append(oraw)
                ses.append(se)
            un = unormp.tile([128, QT], bf16, tag="un", name=f"un{b}{qt}")
            rbs = []
            for h in range(HPC):
                rc = rows.tile([1, QT], f32, tag="rc", name=f"rc{h}")
                nc.vector.reciprocal_approx_fast(rc[:, :], ses[h][:, :])
                rb = rows.tile([64, QT], f32, tag="rb", name=f"rb{h}")
                nc.gpsimd.partition_broadcast(rb[:, :], rc[:, :])
                rbs.append(rb)
            last = (b == B - 1 and qt == T // QT - 1)
            for ts in range(QT // 128):
                c0, c1 = ts * 128, (ts + 1) * 128
                for h in range(HPC):
                    nc.vector.tensor_mul(
                        un[64 * h:64 * h + 64, c0:c1],
                        oraws[h][:, c0:c1], rbs[h][:, c0:c1])
                for ct in range(2):
                    th = th_proj(b, q0, un, ts, ct)
                    if last:  # fine-grained tail: project as soon as ready
                        th()
                    else:
                        projq.append(th)

    while front or projq:
        pop_filler()


def build():
    if "nc" in _CACHE:
        return _CACHE["nc"]
    nc = bacc.Bacc("TRN2", target_bir_lowering=False, debug=False,
                   num_devices=NCORES)
    with tile.TileContext(nc) as tc:
        _emit(tc)
    nc.compile()
    _CACHE["nc"] = nc
    return nc


def make_in_maps(x, qkv_w, qkv_b, proj_w):
    import ml_dtypes
    bf16 = ml_dtypes.bfloat16
    x = np.asarray(x, dtype=np.float32)
    qkv_w = np.asarray(qkv_w, dtype=np.float32)
    qkv_b = np.asarray(qkv_b, dtype=np.float32)
    proj_w = np.asarray(proj_w, dtype=np.float32)

    fp8 = ml_dtypes.float8_e4m3
    # [B,T,C] -> [B, chunk(4), p(128), plane(8), tok(512)] so each chunk
    # DMA has contiguous per-partition runs
    x5 = x.reshape(B, 4, QT, 8, 128).transpose(0, 1, 4, 3, 2)
    xT = np.ascontiguousarray(x5).astype(fp8)
    xTb = np.ascontiguousarray(x5).astype(bf16)
    tri = (np.arange(128)[None, :] >= np.arange(128)[:, None]).astype(bf16)
    ident = np.eye(128, dtype=bf16)

    in_maps = []
    for c in range(NCORES):
        s = 64 * HPC * c  # first feature row of this core's heads
        wq = qkv_w[:, s:s + 128]
        wk = qkv_w[:, C + s:C + s + 128]
        wqk = np.concatenate([wq, wk], axis=1)  # [C, 256]
        # -> [p(128), plane(8), 256] matching the SBUF-resident layout;
        # x16 so the ~+-1/32 weights use fp8e4's normal range (the 1/16
        # de-scales fold into bq and the exp scale)
        wqk = np.ascontiguousarray(
            wqk.reshape(8, 128, 256).transpose(1, 0, 2) * 16.0).astype(fp8)
        wv_c = np.ascontiguousarray(
            qkv_w[:, 2 * C + s:2 * C + s + 128]
            .reshape(8, 128, 128).transpose(1, 0, 2)).astype(bf16)
        bq_c = np.ascontiguousarray(
            16.0 * qkv_b[s:s + 128].reshape(128, 1).astype(np.float32))
        wp_c = np.ascontiguousarray(proj_w[s:s + 128, :]).astype(bf16)
        in_maps.append({
            "xT": xT, "xTb": xTb, "wqkv": wqk, "wv": wv_c, "bq": bq_c,
            "wp": wp_c, "tri": tri, "ident": ident,
        })
    return in_maps


def kernel(x, qkv_w, qkv_b, proj_w, proj_b, _trace=False):
    nc = build()
    in_maps = make_in_maps(x, qkv_w, qkv_b, proj_w)
    res = run_bass_kernel_spmd(nc, in_maps, core_ids=list(range(NCORES)),
                               trace=_trace)
    acc = np.zeros((B, T, C), dtype=np.float64)
    for c in range(NCORES):
        acc += res.results[c]["outp"].astype(np.float64)
    # host-folded bias terms: proj bias + v-bias @ proj_w (exact)
    bv = np.asarray(qkv_b, dtype=np.float64)[2 * C:]
    acc += bv @ np.asarray(proj_w, dtype=np.float64)
    acc += np.asarray(proj_b, dtype=np.float64)
    out = acc.astype(np.float32)
    _CACHE["last_results"] = res
    return out


# revision 26
# speedup vs baseline: 1.1176x; 1.0360x over previous
"""Causal self-attention (B=2, T=2048, C=1024, H=16, D=64) on 8 trn2 cores.

Sharding: tensor-parallel on heads — 2 heads per core. Each core computes
QKV projection for its 2 heads, causal softmax attention, and its heads'
slice of the output projection (a rank-128 partial sum of the full output).
The host pre-transposes x to [B, C, T], slices the weights per core, and
sums the 8 partial outputs (+ proj bias + v-bias@proj_w correction).

Bias algebra (exact):
  - k bias: softmax rows are shift-invariant, so S = (q+bq). k_raw modulo
    per-row constants — the k bias is dropped entirely on device.
  - v bias: rows of normalized attention sum to 1, so attn@(V+1 bv^T) =
    attn@V + 1 bv^T; the bv@proj_w term is a constant row added on host.
  Only the q bias remains on device (fused into the q PSUM->SBUF activation).

Device kernel layout (per core):
  - x^T chunks [128(C), T] stream from DRAM; QKV computed as W^T @ x^T
    giving q/k/v in [feat, tok] layout. All matmul inputs bf16, fp32 accum.
  - Attention q-tiles of 512 tokens. Per k-chunk (128 tokens) the TWO
    heads' S^T matmuls are emitted back-to-back: K=64 contractions at
    base partitions 0/64 auto-derive tile_position (0,0)/(64,0), so they
    run CONCURRENTLY in the two row-halves of the PE array (row tiling).
  - One fused exp ACTIVATE per (q-tile, k-chunk) covers both heads via a
    [128, 2, 512] PSUM pair tile (halves ScalarE instruction overhead).
  - O^T accumulates as (V|1)-chunk.T @ P^T; the ones column makes row 64
    the softmax denominator. Causality: subtile skipping + one [128,128]
    triangle mask multiply per head on the diagonal chunk.
  - Projection: single K=128 matmul per output tile.
  - HAM warm-start: a burst of junk matmuls on the identity tile runs
    during the initial x-DMA wait so real work starts at 2.4 GHz, and a
    filler queue (QKV/V-transpose/projection thunks) keeps the PE dense
    through the ScalarE-paced attention inner loop.
"""

from collections import deque

import numpy as np

import concourse.bass as bass
import concourse.tile as tile
from concourse import bacc, mybir
from concourse.bass_utils import run_bass_kernel_spmd

dt = mybir.dt
AF = mybir.ActivationFunctionType

B, T, C, H, D = 2, 2048, 1024, 16, 64
NCORES = 8
HPC = H // NCORES          # heads per core = 2
QT = 512                   # q-tile (columns of S^T/O^T psum tiles)
KC = 128                   # k chunk (partition dim of S^T)
SCALE = 1.0 / 8.0          # 1/sqrt(D)
NWARM = 40                 # junk matmuls to warm the HAM clock gate

_CACHE = {}


def _emit(tc):
    from contextlib import ExitStack
    with ExitStack() as ctx:
        _emit_body(tc, ctx)


def _emit_body(tc, ctx):
    nc = tc.nc
    f32, bf16 = dt.float32, dt.bfloat16

    fp8 = dt.float8e4
    # x pre-chunked on host as [B, chunk, 128, plane, tok]: per-partition
    # contiguous runs per DMA (vs 256B packets from a strided gather).
    # Q/K projections run as fp8 DoubleRow matmuls (w pre-scaled x16 on
    # host; the 1/16 de-scales fold into bq and the exp scale) — softmax
    # attenuates the logit quantization noise. The V projection stays
    # bf16 (its error passes straight through to the output), using a
    # bf16 copy of x and bf16 weights.
    xT = nc.dram_tensor("xT", [B, 4, 128, 8, QT], fp8,
                        kind="ExternalInput").ap()
    xTb = nc.dram_tensor("xTb", [B, 4, 128, 8, QT], bf16,
                         kind="ExternalInput").ap()
    wqkv = nc.dram_tensor("wqkv", [128, 8, 256], fp8,
                          kind="ExternalInput").ap()
    wv = nc.dram_tensor("wv", [128, 8, 128], bf16,
                        kind="ExternalInput").ap()
    bq = nc.dram_tensor("bq", [128, 1], f32, kind="ExternalInput").ap()
    wp = nc.dram_tensor("wp", [128, C], bf16, kind="ExternalInput").ap()
    tri = nc.dram_tensor("tri", [128, 128], bf16, kind="ExternalInput").ap()
    ident = nc.dram_tensor("ident", [128, 128], bf16, kind="ExternalInput").ap()
    outp = nc.dram_tensor("outp", [B, T, C], bf16, kind="ExternalOutput").ap()

    consts = ctx.enter_context(tc.tile_pool(name="consts", bufs=1))
    xpool = ctx.enter_context(tc.tile_pool(name="xpool", bufs=2))
    qkvpool = ctx.enter_context(tc.tile_pool(name="qkvpool", bufs=6))
    vtmpool = ctx.enter_context(tc.tile_pool(name="vtmpool", bufs=2))
    ptpool = ctx.enter_context(tc.tile_pool(name="ptpool", bufs=4))
    unormp = ctx.enter_context(tc.tile_pool(name="unormp", bufs=3))
    orawp = ctx.enter_context(tc.tile_pool(name="orawp", bufs=4))
    rows = ctx.enter_context(tc.tile_pool(name="rows", bufs=4))
    outsb = ctx.enter_context(tc.tile_pool(name="outsb", bufs=8))
    stp = ctx.enter_context(tc.tile_pool(name="stp", bufs=2, space="PSUM"))
    otp = ctx.enter_context(tc.tile_pool(name="otp", bufs=2, space="PSUM"))
    miscp = ctx.enter_context(tc.tile_pool(name="miscp", bufs=2, space="PSUM"))

    # HAM warm-up: junk matmuls gated only on a local memset so they start
    # right after the engine preamble, covering the whole x-DMA wait.
    wu_in = consts.tile([128, 128], bf16, tag="wuin")
    nc.vector.memset(wu_in, 1.0)
    wu = miscp.tile([128, 128], f32, tag="misc", name="wu")
    for _ in range(NWARM):
        nc.tensor.matmul(wu[:, :], wu_in[:, :], wu_in[:, :],
                         start=True, stop=True)

    # constants / weights resident in SBUF. Order matters: the sync DMA
    # queue serializes, so the first x chunk goes right after the tensors
    # needed by the first QKV chain; wp (first needed by the projection
    # ~30us in) and the remaining x chunks follow.
    tri_sb = consts.tile([128, 128], bf16, tag="tri")
    nc.sync.dma_start(out=tri_sb, in_=tri)
    id_sb = consts.tile([128, 128], bf16, tag="id")
    nc.sync.dma_start(out=id_sb, in_=ident)
    w_sb = consts.tile([128, 8, 256], fp8, tag="w")
    nc.sync.dma_start(out=w_sb, in_=wqkv)
    wv_sb = consts.tile([128, 8, 128], bf16, tag="wv")
    nc.sync.dma_start(out=wv_sb, in_=wv)

    xps = [xpool.tile([128, 4, 8, QT], fp8, tag="xp", name=f"xp{b}")
           for b in range(B)]
    xpbs = [xpool.tile([128, 4, 8, QT], bf16, tag="xpb", name=f"xpb{b}")
            for b in range(B)]

    def dma_x(b, tg):
        nc.sync.dma_start(out=xps[b][:, tg, :, :], in_=xT[b, tg])
        nc.sync.dma_start(out=xpbs[b][:, tg, :, :], in_=xTb[b, tg])

    dma_x(0, 0)
    bq_sb = consts.tile([128, 1], f32, tag="b")
    nc.sync.dma_start(out=bq_sb, in_=bq)
    wp_sb = consts.tile([128, C], bf16, tag="wp")
    nc.sync.dma_start(out=wp_sb, in_=wp)
    for tg in range(1, 4):
        dma_x(0, tg)
    for tg in range(4):
        dma_x(1, tg)

    # dummy exp front-loads the ~1.3us ACT table load
    act_wu = rows.tile([128, 1], f32, tag="actwu")
    nc.scalar.activation(act_wu[:, :], bq_sb[:, :], AF.Exp, scale=SCALE)

    # per-batch q/k/v in [feat, tok] layout and V in token-major layout
    qkv_t = {b: [qkvpool.tile([128, T], bf16, tag="qkv", name=f"qkv{b}_{m}")
                 for m in range(3)] for b in range(B)}
    vt_t = {b: vtmpool.tile([128, 16, HPC * 65], bf16, tag="vtm",
                            name=f"vt{b}")
            for b in range(B)}

    def th_qkv(b, tg, m):
        """One QKV chain: 4 fp8 DoubleRow matmuls + PSUM->SBUF move.
        All PSUM->SBUF moves run on ScalarE (keeps the DVE FIFO clear of
        the attention inner loop's dependencies)."""
        def th():
            pg = miscp.tile([128, QT], f32, tag="misc", name="pg")
            t0 = tg * QT
            if m == 2:  # v: bf16
                for kcw in range(8):
                    nc.tensor.matmul(
                        pg[:, :], wv_sb[:, kcw, :], xpbs[b][:, tg, kcw, :],
                        start=(kcw == 0), stop=(kcw == 7),
                    )
            else:       # q, k: fp8 DoubleRow
                for kcw in range(0, 8, 2):
                    nc.tensor.matmul(
                        pg[:, :],
                        w_sb[:, kcw:kcw + 2, 128 * m:128 * m + 128],
                        xps[b][:, tg, kcw:kcw + 2, :],
                        start=(kcw == 0), stop=(kcw == 6),
                        perf_mode=mybir.MatmulPerfMode.DoubleRow,
                    )
            dst = qkv_t[b][m]
            if m == 0:  # q: fused bias (x16, matching the x16 weights)
                nc.scalar.activation(dst[:, t0:t0 + QT], pg[:, :],
                                     AF.Identity, bias=bq_sb[:, :])
            else:       # k, v: biases dropped (host-folded)
                nc.vector.tensor_copy(out=dst[:, t0:t0 + QT], in_=pg[:, :])
        return th

    def th_ones(b):
        def th():
            nc.vector.memset(
                vt_t[b].rearrange("p k (h c) -> p k h c", h=HPC)[:, :, :, 64:65],
                1.0)
        return th

    def th_vt(b, j0, nj):
        """V [feat,tok] -> token-major [128, j, (h 65)] via PE transposes."""
        def th():
            vT_t, vt = qkv_t[b][2], vt_t[b]
            for j in range(j0, j0 + nj):
                tp = miscp.tile([128, 128], bf16, tag="misc", name="tp")
                nc.tensor.transpose(
                    tp[:, :], vT_t[:, 128 * j:128 * j + 128], id_sb[:, :])
                nc.vector.tensor_copy(
                    out=vt[:, j, :].rearrange(
                        "p (h c) -> p h c", h=HPC)[:, :, 0:64],
                    in_=tp.rearrange("p (h c) -> p h c", h=HPC),
                )
        return th

    def th_proj(b, q0, un, ts, ct, scalar_copy=None):
        """Projection of one [128 tok, 512 feat] output tile + store."""
        def th():
            a0 = q0 + ts * 128
            pp = miscp.tile([128, 512], f32, tag="misc", name="pp")
            nc.tensor.matmul(
                pp[:, :],
                un[:, ts * 128:(ts + 1) * 128],
                wp_sb[:, ct * 512:(ct + 1) * 512],
                start=True, stop=True,
            )
            ob = outsb.tile([128, 512], bf16, tag="osb")
            use_scalar = scalar_copy if scalar_copy is not None else ct == 0
            if use_scalar:  # split PSUM->SBUF copies across ScalarE and DVE
                nc.scalar.copy(ob[:, :], pp[:, :])
            else:
                nc.vector.tensor_copy(out=ob[:, :], in_=pp[:, :])
            nc.sync.dma_start(
                out=outp[b, a0:a0 + 128, ct * 512:(ct + 1) * 512],
                in_=ob[:, :])
        return th

    # front fillers carry (b, tg) tags so attention can force-drain deps
    front = deque()
    projq = deque()
    for b in range(B):
        tgs = range(1, 4) if b == 0 else range(4)
        if b == 1:
            front.append((1, 0, th_ones(1)))
        for tg in tgs:
            front.append((b, tg, th_qkv(b, tg, 1)))   # k
            front.append((b, tg, th_qkv(b, tg, 0)))   # q
            front.append((b, tg, th_qkv(b, tg, 2)))   # v
            front.append((b, tg, th_vt(b, tg * 4, 2)))
            front.append((b, tg, th_vt(b, tg * 4 + 2, 2)))

    slots_left = [2 * sum(4 * (qt + 1) for qt in range(4))]  # 80 kc slots

    def pop_filler():
        # prefer deferred projections; keep front (b1 QKV) as the PE-work
        # reserve for the batch transition where attention runs thin
        if projq:
            projq.popleft()()
        elif front:
            front.popleft()[2]()

    def force_front(b, qt):
        while front and (front[0][0] < b or
                         (front[0][0] == b and front[0][1] <= qt)):
            front.popleft()[2]()

    # batch 0 / tile-group 0 front work runs densely right away
    th_ones(0)()
    th_qkv(0, 0, 1)(); th_qkv(0, 0, 0)(); th_qkv(0, 0, 2)()
    th_vt(0, 0, 2)(); th_vt(0, 2, 2)()

    for b in range(B):
        qT_t, kT_t, vT_t = qkv_t[b]
        vt = vt_t[b]
        for qt in range(T // QT):
            q0 = qt * QT
            nkc = (q0 + QT) // KC
            force_front(b, qt)
            ots = [otp.tile([65, QT], f32, tag="ot", name=f"ot{h}")
                   for h in range(HPC)]
            for kc in range(nkc):
                k0 = kc * KC
                ls = max(0, k0 - q0)
                diag = k0 >= q0
                st = stp.tile([128, HPC, QT], f32, tag="st")
                for h in range(HPC):
                    nc.tensor.matmul(
                        st[:, h, ls:QT],
                        kT_t[64 * h:64 * h + 64, k0:k0 + KC],
                        qT_t[64 * h:64 * h + 64, q0 + ls:q0 + QT],
                        start=True, stop=True,
                    )
                pt = ptpool.tile([128, HPC, QT], bf16, tag="pt")
                nc.scalar.activation(pt[:, :, ls:QT], st[:, :, ls:QT],
                                     AF.Exp, scale=SCALE / 256.0)
                if diag:
                    for h in range(HPC):
                        nc.vector.tensor_mul(
                            pt[:, h, ls:ls + 128], pt[:, h, ls:ls + 128],
                            tri_sb[:, :])
                # start=True only on the first MM into each ot bank (it
                # clears the whole bank's has_written bits); stop=True only
                # on the last emitted MM (kc=nkc-1's diagonal strip).
                for h in range(HPC):
                    vch = vt[:, kc, 65 * h:65 * h + 65]
                    if diag:
                        nc.tensor.matmul(
                            ots[h][:, ls:ls + 128], vch,
                            pt[:, h, ls:ls + 128],
                            start=(kc == 0), stop=(kc == nkc - 1),
                        )
                        if ls + 128 < QT:
                            nc.tensor.matmul(
                                ots[h][:, ls + 128:QT], vch,
                                pt[:, h, ls + 128:QT],
                                start=False, stop=False,
                            )
                    else:
                        nc.tensor.matmul(
                            ots[h][:, 0:QT], vch, pt[:, h, 0:QT],
                            start=(kc == 0), stop=False,
                        )
                slots_left[0] -= 1
                need = len(front) + len(projq)
                npop = min(3, max(1, -(-need // max(slots_left[0], 1))))
                for _ in range(npop):
                    pop_filler()

            last = (b == B - 1 and qt == T // QT - 1)
            if last:
                # fill the PE during the final normalize chain with all
                # remaining deferred work (must be emitted BEFORE the
                # dependent projection matmuls — the PE queue is in-order)
                while front or projq:
                    pop_filler()
            # denominator rows + reciprocals FIRST (they head the critical
            # chain: se -> recip -> broadcast -> mul -> projection), then
            # the ot -> SBUF drains that free the PSUM banks.
            ses, rbs = [], []
            for h in range(HPC):
                se = rows.tile([1, QT], f32, tag="se", name=f"se{h}")
                nc.vector.tensor_copy(out=se[:, :], in_=ots[h][64:65, :])
                ses.append(se)
            rcs = []
            for h in range(HPC):
                rc = rows.tile([1, QT], f32, tag="rc", name=f"rc{h}")
                nc.vector.reciprocal_approx_fast(rc[:, :], ses[h][:, :])
                rcs.append(rc)
            oraws = []
            for h in range(HPC):
                rb = rows.tile([64, QT], f32, tag="rb", name=f"rb{h}")
                nc.gpsimd.partition_broadcast(rb[:, :], rcs[h][:, :])
                rbs.append(rb)
                oraw = orawp.tile([64, QT], f32, tag="oraw", name=f"or{h}")
                nc.vector.tensor_copy(out=oraw[:, :], in_=ots[h][0:64, :])
                oraws.append(oraw)
            un = unormp.tile([128, QT], bf16, tag="un", name=f"un{b}{qt}")
            for ts in range(QT // 128):
                c0, c1 = ts * 128, (ts + 1) * 128
                for h in range(HPC):
                    nc.vector.tensor_mul(
                        un[64 * h:64 * h + 64, c0:c1],
                        oraws[h][:, c0:c1], rbs[h][:, c0:c1])
                for ct in range(2):
                    th = th_proj(b, q0, un, ts, ct,
                                 scalar_copy=True if last else None)
                    if last:  # fine-grained tail: project as soon as ready
                        th()
                    else:
                        projq.append(th)

    while front or projq:
        pop_filler()


def build():
    if "nc" in _CACHE:
        return _CACHE["nc"]
    nc = bacc.Bacc("TRN2", target_bir_lowering=False, debug=False,
                   num_devices=NCORES)
    with tile.TileContext(nc) as tc:
        _emit(tc)
    nc.compile()
    _CACHE["nc"] = nc
    return nc


def make_in_maps(x, qkv_w, qkv_b, proj_w):
    import ml_dtypes
    bf16 = ml_dtypes.bfloat16
    x = np.asarray(x, dtype=np.float32)
    qkv_w = np.asarray(qkv_w, dtype=np.float32)
    qkv_b = np.asarray(qkv_b, dtype=np.float32)
    proj_w = np.asarray(proj_w, dtype=np.float32)

    fp8 = ml_dtypes.float8_e4m3
    # [B,T,C] -> [B, chunk(4), p(128), plane(8), tok(512)] so each chunk
    # DMA has contiguous per-partition runs
    x5 = x.reshape(B, 4, QT, 8, 128).transpose(0, 1, 4, 3, 2)
    xT = np.ascontiguousarray(x5).astype(fp8)
    xTb = np.ascontiguousarray(x5).astype(bf16)
    tri = (np.arange(128)[None, :] >= np.arange(128)[:, None]).astype(bf16)
    ident = np.eye(128, dtype=bf16)

    in_maps = []
    for c in range(NCORES):
        s = 64 * HPC * c  # first feature row of this core's heads
        wq = qkv_w[:, s:s + 128]
        wk = qkv_w[:, C + s:C + s + 128]
        wqk = np.concatenate([wq, wk], axis=1)  # [C, 256]
        # -> [p(128), plane(8), 256] matching the SBUF-resident layout;
        # x16 so the ~+-1/32 weights use fp8e4's normal range (the 1/16
        # de-scales fold into bq and the exp scale)
        wqk = np.ascontiguousarray(
            wqk.reshape(8, 128, 256).transpose(1, 0, 2) * 16.0).astype(fp8)
        wv_c = np.ascontiguousarray(
            qkv_w[:, 2 * C + s:2 * C + s + 128]
            .reshape(8, 128, 128).transpose(1, 0, 2)).astype(bf16)
        bq_c = np.ascontiguousarray(
            16.0 * qkv_b[s:s + 128].reshape(128, 1).astype(np.float32))
        wp_c = np.ascontiguousarray(proj_w[s:s + 128, :]).astype(bf16)
        in_maps.append({
            "xT": xT, "xTb": xTb, "wqkv": wqk, "wv": wv_c, "bq": bq_c,
            "wp": wp_c, "tri": tri, "ident": ident,
        })
    return in_maps


def kernel(x, qkv_w, qkv_b, proj_w, proj_b, _trace=False):
    nc = build()
    in_maps = make_in_maps(x, qkv_w, qkv_b, proj_w)
    res = run_bass_kernel_spmd(nc, in_maps, core_ids=list(range(NCORES)),
                               trace=_trace)
    acc = np.zeros((B, T, C), dtype=np.float64)
    for c in range(NCORES):
        acc += res.results[c]["outp"].astype(np.float64)
    # host-folded bias terms: proj bias + v-bias @ proj_w (exact)
    bv = np.asarray(qkv_b, dtype=np.float64)[2 * C:]
    acc += bv @ np.asarray(proj_w, dtype=np.float64)
    acc += np.asarray(proj_b, dtype=np.float64)
    out = acc.astype(np.float32)
    _CACHE["last_results"] = res
    return out
